# revision 6
# baseline (speedup 1.0000x reference)
"""Trainium2 Bass kernel for DendriticANN (dense_mlp).

Reference computation (fp32):
    h = lrelu(x @ W_in.T + b_in)                        # [B, H]
    for l in 0..L-1:
        dend = lrelu(einsum('bh,ndh->bnd', h, Wd[l]))   # [B, H, D]
        soma = lrelu(einsum('bnd,nd->bn', dend, sd[l])) # [B, H]
        h = lrelu(soma)
    out = h @ W_out.T + b_out                           # [B, OUT]

Strategy: tensor-parallel over the H neuron axis across 8 NeuronCores.
Each core owns 128 neurons; activations live transposed on-chip
(hT = [H partitions, B free]) so every matmul contracts over the
partition dim with no transposes anywhere:

  - input layer (sharded over H): s0_c = lrelu(W_in_c @ x.T + b_in_c)   [128, B]
  - AllGather over cores -> full hT [1024, B]
  - hidden layer per core: for each 128-wide (neuron, dendrite) tile,
      dend^T = WdT_chunk.T @ hT (8 accumulating matmuls, K=128 each)
      s1 = lrelu(dend^T)                                 (ScalarE, alpha=.01)
      soma^T = blockdiag(sd).T @ s1                      (PE does the D-sum,
                                                          sd folded into weights)
      h' = lrelu(lrelu(soma^T)) = lrelu_{1e-4}(soma^T)   (ScalarE, alpha=1e-4)
    -> AllGather -> next hT
  - output layer sharded over OUT rows: outT_c = W_out_c @ hT + b_out_c [125, B]
    (host concatenates the 8 shards; final transpose on host)

Matmuls run in float32r (TF32-like, 1 cyc/row at N>=256 vs 4 for fp32).
"""

import numpy as np

import concourse.bass as bass
import concourse.mybir as mybir
import concourse.tile as tile
from concourse import bacc
from concourse.bass_utils import run_bass_kernel_spmd

# Problem constants (hardcoded per harness contract)
B, IN, H, OUT, L, D = 512, 1024, 1024, 1000, 4, 16
N_CORES = 8
HS = H // N_CORES           # 128 neurons per core
OS = OUT // N_CORES         # 125 output rows per core
KT = H // 128               # 8 k-tiles over the contraction dim
NDT = HS * D // 128         # 16 (neuron,dendrite) tiles of 128 per core
N8 = 128 // D               # 8 neurons per nd-tile

AF = mybir.ActivationFunctionType
F32 = mybir.dt.float32

# matmul dtype: "fp32r" (TF32-like, full speed), "fp32" (exact, 4x slower),
# "bf16" (full speed, half the DMA bytes, lower precision)
MM_DT = "fp32r"

# wd streaming chunk pool depth ([128, 1024] tiles; 4KB/partition in fp32)
WD_BUFS = 20


def _sdt(mm_dt):
    return {
        "fp32r": mybir.dt.float32r,
        "fp32": mybir.dt.float32,
        "bf16": mybir.dt.bfloat16,
    }[mm_dt]


def build_module(mm_dt=None, wd_bufs=None):
    """Build + compile the SPMD Bass module. Returns nc."""
    if mm_dt is None:
        mm_dt = MM_DT
    if wd_bufs is None:
        wd_bufs = WD_BUFS
    sdt = _sdt(mm_dt)
    nc = bacc.Bacc("TRN2", target_bir_lowering=False, debug=False,
                   num_devices=N_CORES)

    # ---- DRAM I/O (per-core shards, host-prepared layouts) ----
    xT_d = nc.dram_tensor("xT", [128, KT, B], sdt, kind="ExternalInput").ap()
    winT_d = nc.dram_tensor("winT", [128, KT, HS], sdt, kind="ExternalInput").ap()
    bin_d = nc.dram_tensor("b_in", [HS, 1], F32, kind="ExternalInput").ap()
    wd_d = nc.dram_tensor("wdT", [L, NDT, 128, KT * 128], sdt,
                          kind="ExternalInput").ap()
    sdb_d = nc.dram_tensor("sdb", [128, L * NDT * N8], sdt,
                           kind="ExternalInput").ap()
    woutT_d = nc.dram_tensor("woutT", [128, KT, OS], sdt,
                             kind="ExternalInput").ap()
    bout_d = nc.dram_tensor("b_out", [OS, 1], F32, kind="ExternalInput").ap()
    outT_d = nc.dram_tensor("outT", [OS, B], F32, kind="ExternalOutput").ap()

    rg = [list(range(N_CORES))]

    with tile.TileContext(nc) as tc:
        with (
            tc.tile_pool(name="const", bufs=1) as cpool,
            tc.tile_pool(name="wd", bufs=wd_bufs) as wdpool,
            tc.tile_pool(name="h", bufs=2) as hpool,
            tc.tile_pool(name="s1p", bufs=4) as s1pool,
            tc.tile_pool(name="soma", bufs=2) as spool,
            tc.tile_pool(name="outp", bufs=1) as opool,
            tc.tile_pool(name="psd", bufs=3, space="PSUM") as ppd,
            tc.tile_pool(name="pss", bufs=3, space="PSUM") as pps,
            tc.tile_pool(name="dram", bufs=2, space="DRAM") as dpool,
        ):
            # ---- persistent loads ----
            xT = cpool.tile([128, KT, B], sdt, name="xT_sb")
            nc.sync.dma_start(xT[:], xT_d[:])
            winT = cpool.tile([128, KT, HS], sdt, name="winT_sb")
            nc.sync.dma_start(winT[:], winT_d[:])
            b_in = cpool.tile([HS, 1], F32, name="bin_sb")
            nc.sync.dma_start(b_in[:], bin_d[:])
            sdb = cpool.tile([128, L * NDT * N8], sdt, name="sdb_sb")
            nc.sync.dma_start(sdb[:], sdb_d[:])
            woutT = cpool.tile([128, KT, OS], sdt, name="woutT_sb")
            nc.sync.dma_start(woutT[:], woutT_d[:])
            b_out = cpool.tile([OS, 1], F32, name="bout_sb")
            nc.sync.dma_start(b_out[:], bout_d[:])

            def gather(agin):
                """AllGather [128,B] core shards -> full hT [128, KT, B] in SBUF."""
                agout = dpool.tile([H, B], sdt, addr_space="Shared",
                                   tag="agout", name="agout")
                nc.gpsimd.collective_compute(
                    "AllGather",
                    mybir.AluOpType.bypass,
                    replica_groups=rg,
                    ins=[agin[:].opt()],
                    outs=[agout[:].opt()],
                )
                hT = hpool.tile([128, KT, B], sdt, tag="hT", name="hT")
                gv = agout[:].rearrange("(kt k) b -> k kt b", k=128)
                nc.sync.dma_start(hT[:], gv)
                return hT

            # ---- input layer (H-sharded) ----
            ps0 = ppd.tile([128, B], F32, tag="pd", name="ps0")
            for kt in range(KT):
                nc.tensor.matmul(ps0[:], winT[:, kt, :], xT[:, kt, :],
                                 start=(kt == 0), stop=(kt == KT - 1))
            s0 = spool.tile([HS, B], sdt, tag="soma", name="s0")
            nc.scalar.activation(s0[:], ps0[:], AF.Lrelu, bias=b_in[:],
                                 alpha=0.01)
            agin0 = dpool.tile([HS, B], sdt, tag="agin", name="agin0")
            nc.sync.dma_start(agin0[:], s0[:])
            hT = gather(agin0)

            # ---- hidden layers ----
            for l in range(L):
                agin = dpool.tile([HS, B], sdt, tag="agin", name=f"agin_l{l}")
                for t in range(NDT):
                    wd_chunk = wdpool.tile([128, KT * 128], sdt, tag="wd",
                                           name=f"wd_l{l}_t{t}")
                    nc.sync.dma_start(wd_chunk[:], wd_d[l, t])
                    psd = ppd.tile([128, B], F32, tag="pd", name=f"pd_l{l}_t{t}")
                    for kt in range(KT):
                        nc.tensor.matmul(
                            psd[:],
                            wd_chunk[:, kt * 128:(kt + 1) * 128],
                            hT[:, kt, :],
                            start=(kt == 0), stop=(kt == KT - 1),
                        )
                    s1 = s1pool.tile([128, B], sdt, tag="s1",
                                     name=f"s1_l{l}_t{t}")
                    nc.scalar.activation(s1[:], psd[:], AF.Lrelu, alpha=0.01)
                    pss_t = pps.tile([N8, B], F32, tag="ps",
                                     name=f"ps_l{l}_t{t}")
                    off = (l * NDT + t) * N8
                    nc.tensor.matmul(pss_t[:], sdb[:, off:off + N8], s1[:],
                                     start=True, stop=True)
                    # h' = lrelu(lrelu(soma)); a single Lrelu with alpha=1e-4
                    # is NOT safe here: the ACT LUT table is shared per
                    # function, so a second alpha silently reuses the first
                    # table. Chain two alpha=0.01 ops instead (identical math).
                    s2a = s1pool.tile([N8, B], sdt, tag="s2a",
                                      name=f"s2a_l{l}_t{t}")
                    nc.scalar.activation(s2a[:], pss_t[:], AF.Lrelu, alpha=0.01)
                    s2 = s1pool.tile([N8, B], sdt, tag="s2",
                                     name=f"s2_l{l}_t{t}")
                    nc.scalar.activation(s2[:], s2a[:], AF.Lrelu, alpha=0.01)
                    nc.sync.dma_start(agin[t * N8:(t + 1) * N8, :], s2[:])
                hT = gather(agin)

            # ---- output layer (OUT-sharded) ----
            pso = ppd.tile([OS, B], F32, tag="pd", name="pso")
            for kt in range(KT):
                nc.tensor.matmul(pso[:], woutT[:, kt, :], hT[:, kt, :],
                                 start=(kt == 0), stop=(kt == KT - 1))
            out_sb = opool.tile([OS, B], F32, name="out_sb")
            nc.scalar.activation(out_sb[:], pso[:], AF.Identity,
                                 bias=b_out[:])
            nc.sync.dma_start(outT_d[:], out_sb[:])

    nc.compile()
    return nc


def _np_dt(mm_dt):
    if mm_dt == "bf16":
        import ml_dtypes
        return np.dtype(ml_dtypes.bfloat16)
    return np.dtype(np.float32)


def make_in_maps(x, W_in, b_in, Wd, sd, W_out, b_out, mm_dt=MM_DT):
    """Host-side sharding/layout prep. Returns per-core input dicts."""
    ndt = _np_dt(mm_dt)
    f32 = np.float32
    x = np.asarray(x, f32)
    W_in = np.asarray(W_in, f32)
    b_in = np.asarray(b_in, f32)
    Wd = np.asarray(Wd, f32)
    sd = np.asarray(sd, f32)
    W_out = np.asarray(W_out, f32)
    b_out = np.asarray(b_out, f32)

    # xT: [k, kt, b] (shared by all cores)
    xT = np.ascontiguousarray(x.reshape(B, KT, 128).transpose(2, 1, 0)).astype(ndt)

    in_maps = []
    for c in range(N_CORES):
        Ws = W_in[c * HS:(c + 1) * HS, :]                      # [128, IN]
        winT = np.ascontiguousarray(
            Ws.reshape(HS, KT, 128).transpose(2, 1, 0)).astype(ndt)
        bin_c = np.ascontiguousarray(b_in[c * HS:(c + 1) * HS, None])

        Wd_c = Wd[:, c * HS:(c + 1) * HS, :, :]                # [L, 128, D, H]
        wdT = np.ascontiguousarray(
            Wd_c.reshape(L, NDT, N8, D, KT, 128).transpose(0, 1, 5, 4, 2, 3)
        ).reshape(L, NDT, 128, KT * 128).astype(ndt)

        sd_c = sd[:, c * HS:(c + 1) * HS, :]                   # [L, 128, D]
        sd_r = sd_c.reshape(L, NDT, N8, D)                     # [l, t, m, d]
        sdb = np.zeros((128, L, NDT, N8), f32)
        for m in range(N8):
            # partition nd = m*D + d gets sd of neuron m in each tile
            sdb[m * D:(m + 1) * D, :, :, m] = sd_r[:, :, m, :].transpose(2, 0, 1)
        sdb = np.ascontiguousarray(sdb.reshape(128, L * NDT * N8)).astype(ndt)

        Wo = W_out[c * OS:(c + 1) * OS, :]                     # [125, H]
        woutT = np.ascontiguousarray(
            Wo.reshape(OS, KT, 128).transpose(2, 1, 0)).astype(ndt)
        bout_c = np.ascontiguousarray(b_out[c * OS:(c + 1) * OS, None])

        in_maps.append({
            "xT": xT,
            "winT": winT,
            "b_in": bin_c,
            "wdT": wdT,
            "sdb": sdb,
            "woutT": woutT,
            "b_out": bout_c,
        })
    return in_maps


_CACHE = {}


def get_module(mm_dt=None, wd_bufs=None):
    if mm_dt is None:
        mm_dt = MM_DT
    if wd_bufs is None:
        wd_bufs = WD_BUFS
    key = (mm_dt, wd_bufs)
    if key not in _CACHE:
        _CACHE[key] = build_module(mm_dt, wd_bufs)
    return _CACHE[key]


def kernel(x, W_in, b_in, Wd, sd, W_out, b_out):
    """Full-input -> full-output entry point (harness contract)."""
    nc = get_module()
    in_maps = make_in_maps(x, W_in, b_in, Wd, sd, W_out, b_out, MM_DT)
    res = run_bass_kernel_spmd(nc, in_maps, core_ids=list(range(N_CORES)))
    out = np.concatenate([res.results[c]["outT"].T for c in range(N_CORES)],
                         axis=1)
    return np.ascontiguousarray(out.astype(np.float32))


# revision 9
# speedup vs baseline: 1.2480x; 1.2480x over previous
"""Trainium2 Bass kernel for DendriticANN (dense_mlp).

Reference computation (fp32):
    h = lrelu(x @ W_in.T + b_in)                        # [B, H]
    for l in 0..L-1:
        dend = lrelu(einsum('bh,ndh->bnd', h, Wd[l]))   # [B, H, D]
        soma = lrelu(einsum('bnd,nd->bn', dend, sd[l])) # [B, H]
        h = lrelu(soma)
    out = h @ W_out.T + b_out                           # [B, OUT]

Strategy: tensor-parallel over the H neuron axis across 8 NeuronCores.
Each core owns 128 neurons; activations live transposed on-chip
(hT = [H partitions, B free]) so every matmul contracts over the
partition dim with no transposes anywhere:

  - input layer (sharded over H): s0_c = lrelu(W_in_c @ x.T + b_in_c)   [128, B]
  - AllGather over cores -> full hT [1024, B]
  - hidden layer per core: for each 128-wide (neuron, dendrite) tile,
      dend^T = WdT_chunk.T @ hT (8 accumulating matmuls, K=128 each)
      s1 = lrelu(dend^T)                                 (ScalarE, alpha=.01)
      soma^T = blockdiag(sd).T @ s1                      (PE does the D-sum,
                                                          sd folded into weights)
      h' = lrelu(lrelu(soma^T)) = lrelu_{1e-4}(soma^T)   (ScalarE, alpha=1e-4)
    -> AllGather -> next hT
  - output layer sharded over OUT rows: outT_c = W_out_c @ hT + b_out_c [125, B]
    (host concatenates the 8 shards; final transpose on host)

Matmuls run in float32r (TF32-like, 1 cyc/row at N>=256 vs 4 for fp32).
"""

import numpy as np

import concourse.bass as bass
import concourse.mybir as mybir
import concourse.tile as tile
from concourse import bacc
from concourse.bass_utils import run_bass_kernel_spmd

# Problem constants (hardcoded per harness contract)
B, IN, H, OUT, L, D = 512, 1024, 1024, 1000, 4, 16
N_CORES = 8
HS = H // N_CORES           # 128 neurons per core
OS = OUT // N_CORES         # 125 output rows per core
KT = H // 128               # 8 k-tiles over the contraction dim
NDT = HS * D // 128         # 16 (neuron,dendrite) tiles of 128 per core
N8 = 128 // D               # 8 neurons per nd-tile

AF = mybir.ActivationFunctionType
F32 = mybir.dt.float32

# matmul dtype: "fp32r" (TF32-like, full speed), "fp32" (exact, 4x slower),
# "bf16" (full speed, half the DMA bytes, lower precision)
MM_DT = "fp32r"

# wd streaming chunk pool depth ([128, 1024] tiles; 4KB/partition in fp32)
WD_BUFS = 20


def _sdt(mm_dt):
    return {
        "fp32r": mybir.dt.float32r,
        "fp32": mybir.dt.float32,
        "bf16": mybir.dt.bfloat16,
    }[mm_dt]


def build_module(mm_dt=None, wd_bufs=None, reps=1):
    """Build + compile the SPMD Bass module. Returns nc.

    reps > 1 unrolls the whole pipeline R times inside one NEFF — used by
    test.py to measure steady-state per-iteration device time via the
    slope between rep counts (no NTFF profiling available under axon).
    """
    if mm_dt is None:
        mm_dt = MM_DT
    if wd_bufs is None:
        wd_bufs = WD_BUFS
    sdt = _sdt(mm_dt)
    nc = bacc.Bacc("TRN2", target_bir_lowering=False, debug=False,
                   num_devices=N_CORES)

    # ---- DRAM I/O (per-core shards, host-prepared layouts) ----
    xT_d = nc.dram_tensor("xT", [128, KT, B], sdt, kind="ExternalInput").ap()
    winT_d = nc.dram_tensor("winT", [128, KT, HS], sdt, kind="ExternalInput").ap()
    bin_d = nc.dram_tensor("b_in", [HS, 1], F32, kind="ExternalInput").ap()
    wd_d = nc.dram_tensor("wdT", [L, NDT, 128, KT * 128], sdt,
                          kind="ExternalInput").ap()
    sdb_d = nc.dram_tensor("sdb", [128, L * NDT * N8], sdt,
                           kind="ExternalInput").ap()
    woutT_d = nc.dram_tensor("woutT", [128, KT, OS], sdt,
                             kind="ExternalInput").ap()
    bout_d = nc.dram_tensor("b_out", [OS, 1], F32, kind="ExternalInput").ap()
    outT_d = nc.dram_tensor("outT", [OS, B], F32, kind="ExternalOutput").ap()

    rg = [list(range(N_CORES))]

    with tile.TileContext(nc) as tc:
        with (
            tc.tile_pool(name="const", bufs=1) as cpool,
            tc.tile_pool(name="wd", bufs=wd_bufs) as wdpool,
            tc.tile_pool(name="h", bufs=2) as hpool,
            tc.tile_pool(name="s1p", bufs=4) as s1pool,
            tc.tile_pool(name="soma", bufs=2) as spool,
            tc.tile_pool(name="outp", bufs=1) as opool,
            tc.tile_pool(name="psd", bufs=3, space="PSUM") as ppd,
            tc.tile_pool(name="pss", bufs=3, space="PSUM") as pps,
            tc.tile_pool(name="dram", bufs=2, space="DRAM") as dpool,
        ):
            # ---- persistent loads ----
            xT = cpool.tile([128, KT, B], sdt, name="xT_sb")
            nc.sync.dma_start(xT[:], xT_d[:])
            winT = cpool.tile([128, KT, HS], sdt, name="winT_sb")
            nc.sync.dma_start(winT[:], winT_d[:])
            b_in = cpool.tile([HS, 1], F32, name="bin_sb")
            nc.sync.dma_start(b_in[:], bin_d[:])
            sdb = cpool.tile([128, L * NDT * N8], sdt, name="sdb_sb")
            nc.sync.dma_start(sdb[:], sdb_d[:])
            woutT = cpool.tile([128, KT, OS], sdt, name="woutT_sb")
            nc.sync.dma_start(woutT[:], woutT_d[:])
            b_out = cpool.tile([OS, 1], F32, name="bout_sb")
            nc.sync.dma_start(b_out[:], bout_d[:])

            def gather(agin):
                """AllGather [128,B] core shards -> full hT [128, KT, B] in SBUF."""
                agout = dpool.tile([H, B], sdt, addr_space="Shared",
                                   tag="agout", name="agout")
                nc.gpsimd.collective_compute(
                    "AllGather",
                    mybir.AluOpType.bypass,
                    replica_groups=rg,
                    ins=[agin[:].opt()],
                    outs=[agout[:].opt()],
                )
                hT = hpool.tile([128, KT, B], sdt, tag="hT", name="hT")
                gv = agout[:].rearrange("(kt k) b -> k kt b", k=128)
                nc.sync.dma_start(hT[:], gv)
                return hT

            def one_pass():
                # ---- input layer (H-sharded) ----
                ps0 = ppd.tile([128, B], F32, tag="pd", name="ps0")
                for kt in range(KT):
                    nc.tensor.matmul(ps0[:], winT[:, kt, :], xT[:, kt, :],
                                     start=(kt == 0), stop=(kt == KT - 1))
                s0 = spool.tile([HS, B], sdt, tag="soma", name="s0")
                nc.scalar.activation(s0[:], ps0[:], AF.Lrelu, bias=b_in[:],
                                     alpha=0.01)
                agin0 = dpool.tile([HS, B], sdt, tag="agin", name="agin0")
                nc.sync.dma_start(agin0[:], s0[:])
                hT = gather(agin0)

                # ---- hidden layers ----
                for l in range(L):
                    agin = dpool.tile([HS, B], sdt, tag="agin",
                                      name=f"agin_l{l}")
                    for t in range(NDT):
                        wd_chunk = wdpool.tile([128, KT * 128], sdt, tag="wd",
                                               name=f"wd_l{l}_t{t}")
                        nc.sync.dma_start(wd_chunk[:], wd_d[l, t])
                        psd = ppd.tile([128, B], F32, tag="pd",
                                       name=f"pd_l{l}_t{t}")
                        for kt in range(KT):
                            nc.tensor.matmul(
                                psd[:],
                                wd_chunk[:, kt * 128:(kt + 1) * 128],
                                hT[:, kt, :],
                                start=(kt == 0), stop=(kt == KT - 1),
                            )
                        s1 = s1pool.tile([128, B], sdt, tag="s1",
                                         name=f"s1_l{l}_t{t}")
                        nc.scalar.activation(s1[:], psd[:], AF.Lrelu,
                                             alpha=0.01)
                        pss_t = pps.tile([N8, B], F32, tag="ps",
                                         name=f"ps_l{l}_t{t}")
                        off = (l * NDT + t) * N8
                        nc.tensor.matmul(pss_t[:], sdb[:, off:off + N8],
                                         s1[:], start=True, stop=True)
                        # h' = lrelu(lrelu(soma)); a single Lrelu with
                        # alpha=1e-4 is NOT safe: the ACT LUT table is shared
                        # per function, so a second alpha silently reuses the
                        # first table. Chain two alpha=0.01 ops instead
                        # (identical math).
                        s2a = s1pool.tile([N8, B], sdt, tag="s2a",
                                          name=f"s2a_l{l}_t{t}")
                        nc.scalar.activation(s2a[:], pss_t[:], AF.Lrelu,
                                             alpha=0.01)
                        s2 = s1pool.tile([N8, B], sdt, tag="s2",
                                         name=f"s2_l{l}_t{t}")
                        nc.scalar.activation(s2[:], s2a[:], AF.Lrelu,
                                             alpha=0.01)
                        nc.sync.dma_start(agin[t * N8:(t + 1) * N8, :], s2[:])
                    hT = gather(agin)

                # ---- output layer (OUT-sharded) ----
                pso = ppd.tile([OS, B], F32, tag="pd", name="pso")
                for kt in range(KT):
                    nc.tensor.matmul(pso[:], woutT[:, kt, :], hT[:, kt, :],
                                     start=(kt == 0), stop=(kt == KT - 1))
                out_sb = opool.tile([OS, B], F32, name="out_sb")
                nc.scalar.activation(out_sb[:], pso[:], AF.Identity,
                                     bias=b_out[:])
                nc.sync.dma_start(outT_d[:], out_sb[:])

            for _rep in range(reps):
                one_pass()

    nc.compile()
    return nc


def _np_dt(mm_dt):
    if mm_dt == "bf16":
        import ml_dtypes
        return np.dtype(ml_dtypes.bfloat16)
    return np.dtype(np.float32)


def make_in_maps(x, W_in, b_in, Wd, sd, W_out, b_out, mm_dt=MM_DT):
    """Host-side sharding/layout prep. Returns per-core input dicts."""
    ndt = _np_dt(mm_dt)
    f32 = np.float32
    x = np.asarray(x, f32)
    W_in = np.asarray(W_in, f32)
    b_in = np.asarray(b_in, f32)
    Wd = np.asarray(Wd, f32)
    sd = np.asarray(sd, f32)
    W_out = np.asarray(W_out, f32)
    b_out = np.asarray(b_out, f32)

    # xT: [k, kt, b] (shared by all cores)
    xT = np.ascontiguousarray(x.reshape(B, KT, 128).transpose(2, 1, 0)).astype(ndt)

    in_maps = []
    for c in range(N_CORES):
        Ws = W_in[c * HS:(c + 1) * HS, :]                      # [128, IN]
        winT = np.ascontiguousarray(
            Ws.reshape(HS, KT, 128).transpose(2, 1, 0)).astype(ndt)
        bin_c = np.ascontiguousarray(b_in[c * HS:(c + 1) * HS, None])

        Wd_c = Wd[:, c * HS:(c + 1) * HS, :, :]                # [L, 128, D, H]
        wdT = np.ascontiguousarray(
            Wd_c.reshape(L, NDT, N8, D, KT, 128).transpose(0, 1, 5, 4, 2, 3)
        ).reshape(L, NDT, 128, KT * 128).astype(ndt)

        sd_c = sd[:, c * HS:(c + 1) * HS, :]                   # [L, 128, D]
        sd_r = sd_c.reshape(L, NDT, N8, D)                     # [l, t, m, d]
        sdb = np.zeros((128, L, NDT, N8), f32)
        for m in range(N8):
            # partition nd = m*D + d gets sd of neuron m in each tile
            sdb[m * D:(m + 1) * D, :, :, m] = sd_r[:, :, m, :].transpose(2, 0, 1)
        sdb = np.ascontiguousarray(sdb.reshape(128, L * NDT * N8)).astype(ndt)

        Wo = W_out[c * OS:(c + 1) * OS, :]                     # [125, H]
        woutT = np.ascontiguousarray(
            Wo.reshape(OS, KT, 128).transpose(2, 1, 0)).astype(ndt)
        bout_c = np.ascontiguousarray(b_out[c * OS:(c + 1) * OS, None])

        in_maps.append({
            "xT": xT,
            "winT": winT,
            "b_in": bin_c,
            "wdT": wdT,
            "sdb": sdb,
            "woutT": woutT,
            "b_out": bout_c,
        })
    return in_maps


_CACHE = {}


def get_module(mm_dt=None, wd_bufs=None):
    if mm_dt is None:
        mm_dt = MM_DT
    if wd_bufs is None:
        wd_bufs = WD_BUFS
    key = (mm_dt, wd_bufs)
    if key not in _CACHE:
        _CACHE[key] = build_module(mm_dt, wd_bufs)
    return _CACHE[key]


def kernel(x, W_in, b_in, Wd, sd, W_out, b_out):
    """Full-input -> full-output entry point (harness contract)."""
    nc = get_module()
    in_maps = make_in_maps(x, W_in, b_in, Wd, sd, W_out, b_out, MM_DT)
    res = run_bass_kernel_spmd(nc, in_maps, core_ids=list(range(N_CORES)))
    out = np.concatenate([res.results[c]["outT"].T for c in range(N_CORES)],
                         axis=1)
    return np.ascontiguousarray(out.astype(np.float32))
